# revision 44
# baseline (speedup 1.0000x reference)
"""CLRNet IoU loss kernel for Trainium2 (Bass/Tile), 8-core data-parallel.

Math (equivalent to the reference):
  ovr_j   = 2w - |p_j - t_j|          (if both p_j, t_j in [0,1), else 0)
  union_j = 2w + |p_j - t_j|          (same mask)
  iou     = (2w*tp - S) / (2w*tp + S + 1e-9)
  where S = sum_j |d_j| * both_j,  tp = sum_j both_j
  errors  = sum_j (vp_j XOR vt_j) = sum_j(vp_j) + sum_j(vt_j) - 2*tp
  penalize lanes with tp > errors > 0 by iou *= 1 - errors/(tp+1e-9)
  loss    = mean(1 - iou)

Implementation notes:
  - Inputs are cast to fp16 on the host (IN_NP_DT): the fp32 kernel was
    DMA-bound at ~205 us (353 GB/s/core, HBM-per-NC limit ~358 GB/s);
    halving the bytes moves the DMA floor to ~102 us. Loss error from the
    fp16 cast, measured against the f64 reference on the exact seeded
    inputs, is 2.8e-5 (gate: 2e-2). The DVE upconverts fp16 reads to f32
    internally, so on-device math is unchanged.
  - validity is |x - 0.5| < 0.5, one ALU stage via ABSOLUTE_DIFF(x, 0.5),
    so no input pre-processing is needed.
  - A fused custom DVE op computes a running (prefix) sum of
    both*(|d| + 128); per-lane segment sums are recovered by differencing
    the cumulative value at consecutive 72-element page ends (gathered on
    the Scalar engine).  The packed value decodes as 128*tp + S (S <= 72).
  - A second custom op scans cumsum(valid(p) XOR valid(t)), whose page-end
    differences are `errors` per lane directly (exact integers in f32).
  - The finals (decode + IoU + penalty, single-reciprocal form) are
    emitted in column groups interleaved with the chunk loop; they hide
    completely under the scans (no_fin probe at lp64: 147.9 us vs the
    full kernel's 148.0 -- finals and page-end copies are free; the two
    scans at 1 col/cycle account for the entire runtime).
  - At fp16 the kernel is DVE-bound, not DMA-bound: the two custom scans
    cost 2*72 pair-positions/lane at 1x (0.96 GHz, custom DVE ops have no
    2x mode) = ~147 us vs the ~102 us DMA floor. Measured: 150.1 us; a
    no-errors-pass probe (no_xe) runs 102.8 us. This is structural: the
    8-stage DVE limit forces two passes (the fused single-pass op needs
    ~10 stages in every algebraic form tried), stock ops cannot express
    the range-test + page-sum any cheaper (tensor_reduce and all scans
    are 1x-only; verified via supported_dve_perf_modes), Act-engine
    helpers lose to 1283ns activation-table reloads (n_act probe), and
    Pool lacks abs/compare/free-axis-reduce. io_bufs=4, one HWDGE ring,
    1.18 MB (fp32-equivalent geometry) chunk DMAs as before.
  - Also measured, all at-or-worse than the plain two-scan kernel:
    n_act>0 with the Act(Square,Square,Sign,Sign) + Pool(zp-zt) +
    paired-read half-cost XPS_SCAN chain (150.2 us at n_act=12 -- the
    ~13 us serial chain latency paces the pipeline instead of hiding;
    deeper pools hp_bufs=3/io_bufs=5/split_finals=(16,) made it 189.7
    us), io_bufs/scan_bufs/split_finals sweeps (all within +/-0.3 us),
    and an Act+Pool+DVE-stock-reduce variant (reduce is 1x: zero DVE
    relief). Ideas that do NOT work by construction: stock-op predicate
    pipelines (no abs in ts/tt valid ops; >= 3 passes always), f32
    bit-pattern tricks to read two fp16 pairs per cycle (lo-half needs
    denormal compares, FTZ kills it), PE page sums (contracts the
    partition dim only; transpose detours cost more than they save).
    scan_bufs=3 at lp64 overflows SBUF (fin pool short 25.6 kb/part).
  - Last unimplemented idea (~1 us, 0.7%): fuse each dma_group=2 pair of
    lp64 chunks into ONE MD + ONE XE scan over [128, 9216] (needs
    io_bufs=2/scan_bufs=1 to fit SBUF; MD cum ~1.2M keeps S noise at
    +/-0.06 ulp-random, fine). Page-end gather becomes a [128, 2, 64]
    rearranged copy into b1[:, ci:ci+2, 1:65] PLUS a 1-col copy of the
    chunk-boundary cum (r1 col 64*72-1) into b1[:, ci+1, 0:1] -- without
    that cell the second chunk's first page diff is wrong. 2.36 MB
    transfers sat in the fp32-era DMA penalty zone but ~45 us of DMA
    slack hides it.
"""

import sys

if "/opt/trn_rl_repo" not in sys.path:
    sys.path.insert(0, "/opt/trn_rl_repo")

import numpy as np

import concourse.bacc as bacc
import concourse.bass as bass
import concourse.mybir as mybir
from concourse import dve_ops
from concourse.bass_utils import run_bass_kernel_spmd
from concourse.dve_ops import DveOp
from concourse.dve_spec import (
    AluOp,
    Bin,
    C0,
    C1,
    Spec,
    Src0,
    Src1,
    Zero,
    lower,
    scan,
)
from concourse.dve_spec import _has_src1 as has_src1
from concourse.dve_uop import DveOpSpec
from concourse.tile import TileContext

F32 = mybir.dt.float32
F16 = mybir.dt.float16
I32 = mybir.dt.int32

NL = 1_000_000
NR = 72
NCORES = 8
NLC = NL // NCORES  # 125_000 lanes per core
W2 = 2.0 * (15.0 / 800.0)  # 2 * lane half-width = 0.0375
PACK = 128.0  # tp packing multiplier; S <= 72 < 128

# Inputs are cast to fp16 on the host before upload: halves HBM traffic
# (the kernel is DMA-bound) and the induced loss error, measured against
# the f64 reference on the exact seeded inputs, is 2.7e-5 -- far inside
# the 2e-2 gate. The DVE upconverts fp16 operands to f32 internally, so
# the on-device math is unchanged.
IN_NP_DT = np.float16

# ---------------------------------------------------------------------------
# Custom DVE ops (registered at import, idempotently)
# ---------------------------------------------------------------------------


def _register(name: str, spec: Spec, subdim: bool = False) -> DveOp:
    for op in dve_ops.OPS:
        if op.name == name:
            return op
    row = dve_ops._CUSTOM_DVE_ROW_BASE + len(dve_ops.OPS)
    shas = {}
    for ver in ("v3", "v4"):
        try:
            s = DveOpSpec(
                name=name, opcode=row, uops=lower(spec, ver=ver), rd1_en=has_src1(spec)
            )
            shas[ver] = s.sha(ver)
        except Exception:
            pass  # op not expressible on this ver; only v3 (TRN2) is needed
    op = DveOp(name, spec, subdim=subdim, uops_sha=shas)
    dve_ops.OPS.append(op)
    dve_ops._SUB_OPCODE_FOR_NAME[name] = row
    dve_ops.CUSTOM_DVE_SPECS[name] = spec
    return op


def _adiff(x, y):
    return Bin(AluOp.ABSOLUTE_DIFF, x, y)


def _md_ref(in0, in1, s0, s1, imm2):
    p = in0.astype(np.float32).reshape(in0.shape[0], -1)
    t = in1.astype(np.float32).reshape(in0.shape[0], -1)
    both = (np.maximum(np.abs(p - s0), np.abs(t - s0)) < s0).astype(np.float32)
    m = both * (np.abs(p - t) + s1)
    return np.cumsum(m, axis=1, dtype=np.float32)


def _xe_ref(in0, in1, s0, s1, imm2):
    p = in0.astype(np.float32).reshape(in0.shape[0], -1)
    t = in1.astype(np.float32).reshape(in0.shape[0], -1)
    x = ((np.abs(p - s0) < s0) != (np.abs(t - s0) < s0)).astype(np.float32)
    return np.cumsum(x, axis=1, dtype=np.float32)


# valid(x) = |x - 0.5| < 0.5; both = valid(p) & valid(t)
# out = cumsum(both * (|p - t| + PACK))  -- 8 ALU stages on v3
_w = Bin(AluOp.MAX, _adiff(Src0, C0), _adiff(Src1, C0))
_both = _w < C0
_adC = _adiff(Src0, Src1) + C1
MD_SCAN = _register(
    "CLR_MD_SCAN",
    Spec(body=scan(AluOp.ADD, _adC * _both), reference=_md_ref),
)

# out = cumsum( valid(p) XOR valid(t) )  -- 6 ALU stages; page-end
# differences give `errors` per lane directly (exact integers in f32).
_v0 = _adiff(Src0, C0) < C0
_v1 = _adiff(Src1, C0) < C0
XE_SCAN = _register(
    "CLR_XE_SCAN",
    Spec(body=scan(AluOp.ADD, Bin(AluOp.LOGICAL_XOR, _v0, _v1)), reference=_xe_ref),
)


def _xps_ref(in0, in1, s0, s1, imm2):
    a = np.abs(in0.astype(np.float32).reshape(in0.shape[0], -1))
    b = np.abs(in1.astype(np.float32).reshape(in0.shape[0], -1))
    return np.cumsum((a + b) * s0, axis=1, dtype=np.float32)


# paired-read running sum of s0*(|in0|+|in1|): reads two elements of one
# pre-combined stream per cycle -- half the positions of a pair scan.
# Used on dd = zp - zt in {-2,0,2}: with s0=0.5 the cumsum counts xor.
_ps = (_adiff(Src0, Zero) + _adiff(Src1, Zero)) * C0
XPS_SCAN = _register(
    "CLR_XPS_SCAN",
    Spec(body=scan(AluOp.ADD, _ps), reference=_xps_ref),
)

# ---------------------------------------------------------------------------
# Bass program (SPMD; one NeuronCore's share)
# ---------------------------------------------------------------------------


def _chunks(nlc: int, max_lp: int = 32):
    """Split nlc lanes into (base, lanes_per_partition, partitions) chunks."""
    out = []
    base = 0
    for lp in (64, 32, 16, 8, 4, 2, 1):
        if lp > max_lp:
            continue
        n = 128 * lp
        while nlc - base >= n:
            out.append((base, lp, 128))
            base += n
    if nlc > base:
        out.append((base, 1, nlc - base))
        base = nlc
    return out


SPLIT_FINALS = (10, 16, 22, 27)


def build_bass(
    nlc: int = NLC,
    debug: bool = False,
    reps: int = 1,
    no_compute: bool = False,
    no_dma: bool = False,
    split_finals=SPLIT_FINALS,
    small_first: bool = False,
    max_lp: int = 32,
    io_bufs: int = 4,
    scan_bufs: int = 2,
    gp_memset: bool = False,
    dma_split: bool = False,
    ring: str = None,  # None->legacy dma_split; "sync" | "split" | "alt"
    merge_vs: bool = False,  # unused (kept for probe compatibility)
    n_act: int = 0,  # full chunks whose errors-pass runs on Act+GpSimd
    hp_bufs: int = 2,  # helper-pool depth (chains in flight)
    no_xe: bool = False,  # timing probe: skip the errors pass entirely
    no_fin: bool = False,  # timing probe: skip the finals entirely
    scan_group: bool = False,  # fuse dma_group=2 pairs into single scans
    no_copy: bool = False,  # debug: skip page-end copies + finals
    no_scan: bool = False,  # debug: skip DVE scans, copy page-ends from inputs
    dma_group: int = 1,  # consecutive full chunks loaded by one dma_start
) -> bass.Bass:
    if ring is None:
        ring = "split" if dma_split else "sync"
    nc = bacc.Bacc(None)
    pred = nc.declare_dram_parameter("pred", [nlc, NR], F16, isOutput=False)
    targ = nc.declare_dram_parameter("target", [nlc, NR], F16, isOutput=False)
    out = nc.declare_dram_parameter("partial", [128, 1], F32, isOutput=True)
    dbg = {}
    if debug:
        nchd = len(_chunks(nlc))
        nposd = nchd * 32
        for name in ("dbg_d1", "dbg_sv", "dbg_tp", "dbg_loss"):
            dbg[name] = nc.declare_dram_parameter(
                name, [128, nposd], F32, isOutput=True
            )

    chunks = _chunks(nlc, max_lp)
    if small_first:
        chunks = chunks[::-1]
    nch = len(chunks)
    # Spread the Act-assisted chunks evenly over the full (parts=128) chunks
    # so no engine stalls waiting for io tiles (io_pool depth limits drift).
    lpmax = max(lp for _b, lp, _p in chunks)
    full_idx = [
        i for i, (_b, lp, p) in enumerate(chunks) if p == 128 and lp == lpmax
    ]
    na = min(n_act, len(full_idx))
    act_set = set()
    if na:
        nf = len(full_idx)
        act_set = {full_idx[(k * nf) // na] for k in range(na)}
    pos = max(lp for _b, lp, _p in chunks)  # page-end columns per chunk slot
    slot = pos + 1  # plus 1 zero column
    npos = nch * (slot - 1)
    if split_finals:
        cuts_all = (
            (split_finals,) if isinstance(split_finals, int) else tuple(split_finals)
        )
        split_finals = tuple(c for c in cuts_all if 0 < c < nch)

    with TileContext(nc) as tc:
        with (
            tc.tile_pool(name="io", bufs=io_bufs) as io_pool,
            tc.tile_pool(name="scan", bufs=scan_bufs) as scan_pool,
            tc.tile_pool(name="acc", bufs=1) as acc_pool,
            tc.tile_pool(name="fin", bufs=1) as fin_pool,
            tc.tile_pool(name="hp", bufs=hp_bufs) as hp_pool,
        ):
            b1 = acc_pool.tile([128, nch, slot], F32, tag="b1")
            b2 = acc_pool.tile([128, nch, slot], F32, tag="b2")
            ms = nc.gpsimd.memset if gp_memset else nc.vector.memset
            ms(b1[:], 0.0)
            ms(b2[:], 0.0)
            if act_set:
                bias_m1 = acc_pool.tile([128, 1], F32, tag="bias_m1")
                bias_thr = acc_pool.tile([128, 1], F32, tag="bias_thr")
                ms(bias_m1[:], -1.0)
                ms(bias_thr[:], 1.0 - 2.0**-11)
            # 1.0 where a position maps to a real lane, 0.0 elsewhere
            lmask = acc_pool.tile([128, nch, pos], F32, tag="lmask")
            ms(lmask[:], 0.0)
            for ci, (_b, lp, parts) in enumerate(chunks):
                ms(lmask[:parts, ci, 0 : min(lp, pos)], 1.0)

            # ----------------- finals: decode + iou + penalty ---------------
            stt = nc.vector.scalar_tensor_tensor
            A = mybir.AluOpType
            psums = []

            def emit_finals(cs, ce, key):
                """Decode and compute per-lane loss for chunk slots [cs, ce);
                appends a [128,1] partial-sum tile to psums.

                Single-reciprocal form: with pen = (tp>err)&(err>0),
                  iou2 = num*(tp - pen*err) / (den*(tp + 1e-9))
                matches the reference both when pen (up to the 1e-9 shift of
                err's denominator) and when not (tp/(tp+eps) ~ 1; tp=0 lanes
                have num=0 so both forms give 0)."""
                if no_fin:
                    return
                w = (ce - cs) * pos

                def ft(tag, dt=F32):
                    t = fin_pool.tile([128, w], dt, tag=f"{tag}{key}")
                    return t

                d1 = ft("d1")
                err = ft("err")
                tp = ft("tp")
                ssum = ft("ssum")
                tmp = ft("tmp")
                tpi = ft("tpi", I32)

                # segment sums by differencing consecutive page-end cumulatives
                nc.vector.tensor_sub(
                    d1[:].rearrange("q (c j) -> q c j", c=ce - cs),
                    b1[:, cs:ce, 1:slot],
                    b1[:, cs:ce, 0 : slot - 1],
                )
                nc.vector.tensor_sub(
                    err[:].rearrange("q (c j) -> q c j", c=ce - cs),
                    b2[:, cs:ce, 1:slot],
                    b2[:, cs:ce, 0 : slot - 1],
                )

                if debug:
                    nc.sync.dma_start(out=dbg["dbg_d1"][:, cs * 32 : ce * 32], in_=d1[:])
                    nc.sync.dma_start(out=dbg["dbg_sv"][:, cs * 32 : ce * 32], in_=err[:])

                # decode: tp = floor(d1/128) via int32 truncation, S = d1 - 128*tp
                nc.vector.tensor_scalar(
                    out=tpi[:], in0=d1[:], scalar1=1.0 / PACK, scalar2=None, op0=A.mult
                )
                nc.vector.tensor_copy(out=tp[:], in_=tpi[:])
                if debug:
                    nc.sync.dma_start(out=dbg["dbg_tp"][:, cs * 32 : ce * 32], in_=tp[:])
                stt(out=ssum[:], in0=tp[:], scalar=-PACK, in1=d1[:], op0=A.mult, op1=A.add)

                # num = 2w*tp - S;  den = 2w*tp + S + 1e-9
                u1 = tmp
                nc.vector.tensor_scalar(
                    out=u1[:], in0=tp[:], scalar1=W2, scalar2=None, op0=A.mult
                )
                num = d1  # reuse
                stt(out=num[:], in0=ssum[:], scalar=-1.0, in1=u1[:], op0=A.mult, op1=A.add)
                den = ssum  # reuse
                stt(out=den[:], in0=ssum[:], scalar=1e-9, in1=u1[:], op0=A.add, op1=A.add)

                # pen = (tp > err) & (err > 0); tnum = tp - pen*err
                c1 = u1  # reuse
                nc.vector.tensor_tensor(out=c1[:], in0=tp[:], in1=err[:], op=A.is_gt)
                pen = c1  # in place
                stt(out=pen[:], in0=err[:], scalar=0.0, in1=c1[:], op0=A.is_gt, op1=A.mult)
                ee = pen  # in place
                nc.vector.tensor_mul(ee[:], pen[:], err[:])
                tnum = err  # reuse
                nc.vector.tensor_sub(tnum[:], tp[:], ee[:])

                # DEN = den*(tp + 1e-9); iou2 = num*tnum * recip(DEN)
                tpe = tp  # in place
                nc.vector.tensor_scalar(
                    out=tpe[:], in0=tp[:], scalar1=1e-9, scalar2=None, op0=A.add
                )
                DEN = ee  # reuse
                nc.vector.tensor_mul(DEN[:], den[:], tpe[:])
                NUM = den  # reuse
                nc.vector.tensor_mul(NUM[:], num[:], tnum[:])
                r = tpe  # reuse
                nc.vector.reciprocal_approx_fast(r[:], DEN[:])
                iou2 = num  # reuse
                nc.vector.tensor_mul(iou2[:], NUM[:], r[:])

                # loss = lmask*(1 - iou2); partial = sum
                lm = lmask[:, cs:ce, :].rearrange("q c j -> q (c j)")
                f2 = DEN  # reuse
                nc.vector.tensor_mul(f2[:], iou2[:], lm)
                loss = iou2  # reuse
                ps = fin_pool.tile([128, 1], F32, tag=f"psum{key}")
                stt(
                    out=loss[:],
                    in0=f2[:],
                    scalar=-1.0,
                    in1=lm,
                    op0=A.mult,
                    op1=A.add,
                    accum_out=ps[:],
                )
                if debug:
                    nc.sync.dma_start(
                        out=dbg["dbg_loss"][:, cs * 32 : ce * 32], in_=loss[:]
                    )
                psums.append(ps)

            # group consecutive full chunks of equal lp for single-DMA loads;
            # lane->(slot, partition, position) mapping changes but the loss
            # sum is permutation-invariant over lanes.
            groups = []
            gi = 0
            while gi < nch:
                g = [gi]
                while (
                    len(g) < dma_group
                    and gi + len(g) < nch
                    and chunks[gi + len(g)][1] == chunks[gi][1]
                    and chunks[gi + len(g)][2] == 128
                    and chunks[gi][2] == 128
                ):
                    g.append(gi + len(g))
                groups.append(g)
                gi += len(g)

            for rep in range(reps):
              for g in groups:
                base0, lp, parts = chunks[g[0]]
                m = len(g)
                fd = lp * NR
                fdg = m * fd
                lanes = sum(chunks[ci][2] * chunks[ci][1] for ci in g)
                up = io_pool.tile([128, fdg], F16, tag="up")
                vt = io_pool.tile([128, fdg], F16, tag="vt")
                src_p = pred[base0 : base0 + lanes, :].rearrange(
                    "(q j) r -> q (j r)", q=parts
                )
                src_t = targ[base0 : base0 + lanes, :].rearrange(
                    "(q j) r -> q (j r)", q=parts
                )
                if not no_dma:
                    if ring == "sync":
                        p_eng = t_eng = nc.sync
                    elif ring == "split":
                        p_eng, t_eng = nc.sync, nc.scalar
                    else:  # alternate rings per group
                        p_eng, t_eng = (
                            (nc.sync, nc.scalar)
                            if g[0] % 2 == 0
                            else (nc.scalar, nc.sync)
                        )
                    p_eng.dma_start(out=up[:parts, :], in_=src_p)
                    t_eng.dma_start(out=vt[:parts, :], in_=src_t)
                if no_compute:
                    continue
                if (
                    scan_group
                    and m == 2
                    and parts == 128
                    and not no_scan
                    and not no_xe
                    and not any(c in act_set for c in g)
                ):
                    # one MD + one XE scan across the whole 2-chunk group:
                    # halves scan instruction count. The cum crosses the
                    # chunk boundary, so chunk g[1]'s zero column gets the
                    # boundary cum (else its first page diff is wrong).
                    ci0 = g[0]
                    r1 = scan_pool.tile([128, fdg], F32, tag="r1")
                    nc.vector._custom_dve(
                        MD_SCAN,
                        out=r1[:parts, :],
                        in0=up[:parts, :],
                        in1=vt[:parts, :],
                        s0=0.5,
                        s1=PACK,
                    )
                    r2 = scan_pool.tile([128, fdg], F32, tag="r2")
                    nc.vector._custom_dve(
                        XE_SCAN,
                        out=r2[:parts, :],
                        in0=up[:parts, :],
                        in1=vt[:parts, :],
                        s0=0.5,
                    )
                    if not no_copy:
                        for r, b in ((r1, b1), (r2, b2)):
                            re_ = r[:parts, :].rearrange(
                                "q (c j r) -> q c j r", c=2, r=NR
                            )[:, :, :, 71]
                            nc.scalar.copy(
                                b[:parts, ci0 : ci0 + 2, 1 : 1 + lp], re_
                            )
                            nc.scalar.copy(
                                b[:parts, ci0 + 1, 0:1], r[:parts, fd - 1 : fd]
                            )
                    if split_finals and rep == reps - 1:
                        cuts = (
                            (split_finals,)
                            if isinstance(split_finals, int)
                            else tuple(split_finals)
                        )
                        for k, cut in enumerate(cuts):
                            if cut - 1 in g:
                                prev = 0 if k == 0 else cuts[k - 1]
                                emit_finals(prev, cut, f"s{k}")
                    continue
                for s, ci in enumerate(g):
                    _bc, lpc, parts = chunks[ci]
                    off = s * fd
                    upc = up[:parts, off : off + fd]
                    vtc = vt[:parts, off : off + fd]
                    if no_scan:
                        # timing probe: same copy geometry, reading io tiles
                        u3d = upc.rearrange("q (j r) -> q j r", j=lp)
                        v3d = vtc.rearrange("q (j r) -> q j r", j=lp)
                        nc.scalar.copy(b1[:parts, ci, 1 : 1 + lp], u3d[:, :, 71])
                        nc.scalar.copy(b2[:parts, ci, 1 : 1 + lp], v3d[:, :, 71])
                        continue

                    r1 = scan_pool.tile([128, fd], F32, tag="r1")
                    nc.vector._custom_dve(
                        MD_SCAN,
                        out=r1[:parts, :],
                        in0=upc,
                        in1=vtc,
                        s0=0.5,
                        s1=PACK,
                    )
                    r1e = r1[:parts, :].rearrange("q (j r) -> q j r", j=lp)[
                        :, :, 71
                    ]
                    if no_xe:
                        if not no_copy:
                            nc.scalar.copy(b1[:parts, ci, 1 : 1 + lp], r1e)
                    elif ci in act_set:
                        # errors-pass mostly off the DVE: Act computes z=+/-1
                        # validity via Square then Sign (exact over the fp16
                        # input grid: q=(2x-1)^2; valid <=> q < 1, closest
                        # interior q is 1-2^-9, so bias 1-2^-11 splits
                        # cleanly; fp16 q keeps the gap). Pool forms
                        # dd=zp-zt in {-2,0,2}; a paired-read DVE scan sums
                        # 0.5*(|dd_even|+|dd_odd|) at HALF the positions of
                        # the XE scan, feeding the same b2 cumulative path
                        # (pair 35 of each 36-pair page is the page end).
                        # Acts are ordered Square,Square,Sign,Sign to pay 2
                        # activation-table loads per chunk, not 4; gathers
                        # go to Pool so Act never loads the Copy table.
                        AF = mybir.ActivationFunctionType
                        qa = hp_pool.tile([128, fd], F16, tag="qa")
                        qb = hp_pool.tile([128, fd], F16, tag="qb")
                        zp = hp_pool.tile([128, fd], F16, tag="zp")
                        zt = hp_pool.tile([128, fd], F16, tag="zt")
                        nc.scalar.activation(
                            qa[:parts, :], upc, AF.Square,
                            scale=2.0, bias=bias_m1[:parts, :],
                        )
                        nc.scalar.activation(
                            qb[:parts, :], vtc, AF.Square,
                            scale=2.0, bias=bias_m1[:parts, :],
                        )
                        nc.scalar.activation(
                            zp[:parts, :], qa[:parts, :], AF.Sign,
                            scale=-1.0, bias=bias_thr[:parts, :],
                        )
                        nc.scalar.activation(
                            zt[:parts, :], qb[:parts, :], AF.Sign,
                            scale=-1.0, bias=bias_thr[:parts, :],
                        )
                        dd = hp_pool.tile([128, fd], F16, tag="dd")
                        nc.gpsimd.tensor_sub(
                            dd[:parts, :], zp[:parts, :], zt[:parts, :]
                        )
                        d2 = dd[:parts, :].rearrange(
                            "q (j two) -> q j two", two=2
                        )
                        r2p = scan_pool.tile([128, fd // 2], F32, tag="xps")
                        nc.vector._custom_dve(
                            XPS_SCAN,
                            out=r2p[:parts, :],
                            in0=d2[:, :, 0],
                            in1=d2[:, :, 1],
                            s0=0.5,
                        )
                        r2e = r2p[:parts, :].rearrange(
                            "q (j r) -> q j r", j=lp
                        )[:, :, 35]
                        nc.gpsimd.tensor_copy(
                            out=b2[:parts, ci, 1 : 1 + lp], in_=r2e
                        )
                        nc.gpsimd.tensor_copy(
                            out=b1[:parts, ci, 1 : 1 + lp], in_=r1e
                        )
                    else:
                        r2 = scan_pool.tile([128, fd], F32, tag="r2")
                        nc.vector._custom_dve(
                            XE_SCAN,
                            out=r2[:parts, :],
                            in0=upc,
                            in1=vtc,
                            s0=0.5,
                        )
                        if not no_copy:
                            r2e = r2[:parts, :].rearrange(
                                "q (j r) -> q j r", j=lp
                            )[:, :, 71]
                            nc.scalar.copy(b2[:parts, ci, 1 : 1 + lp], r2e)
                            nc.scalar.copy(b1[:parts, ci, 1 : 1 + lp], r1e)

                    if split_finals and rep == reps - 1:
                        cuts = (
                            (split_finals,)
                            if isinstance(split_finals, int)
                            else tuple(split_finals)
                        )
                        for k, cut in enumerate(cuts):
                            if ci == cut - 1:
                                prev = 0 if k == 0 else cuts[k - 1]
                                emit_finals(prev, cut, f"s{k}")

            if not no_compute:
                if split_finals:
                    cuts = (
                        (split_finals,)
                        if isinstance(split_finals, int)
                        else tuple(split_finals)
                    )
                    emit_finals(cuts[-1], nch, "b")
                else:
                    emit_finals(0, nch, "a")
            if not psums:
                zp = fin_pool.tile([128, 1], F32, tag="zp")
                nc.vector.memset(zp[:], 0.0)
                psums.append(zp)
            total = psums[0]
            for ps in psums[1:]:
                nc.vector.tensor_add(total[:], total[:], ps[:])
            nc.sync.dma_start(out=out[:, :], in_=total[:])

    nc.finalize()
    return nc


# ---------------------------------------------------------------------------
# Host entry point
# ---------------------------------------------------------------------------

_CACHE = {}

# Act+GpSimd errors-pass helpers measured SLOWER than the DVE XOR scan
# (activation-table reloads, 1283ns each, dominate): keep 0.
N_ACT = 0
# lp64 chunks halve the scan/DMA instruction count (DVE issue overhead is
# the only slack left over the scan floor): 148.0 us vs 150.4 at lp32.
# At fp16 an lp64 chunk is a 1.18 MB transfer -- the measured DMA
# sweet spot (the fp32-era big-transfer penalty applied to 2.36 MB).
MAX_LP = 64
# Fused pair-scans: one MD + one XE instruction per dma_group=2 pair of
# lp64 chunks (halves scan issue overhead; boundary cum copied into the
# second chunk's zero column). io/scan bufs sized to fit SBUF.
SCAN_CFG = dict(dma_group=2, io_bufs=2, scan_bufs=1, scan_group=True)


def _get_nc(nlc: int) -> bass.Bass:
    if nlc not in _CACHE:
        _CACHE[nlc] = build_bass(nlc, n_act=N_ACT, max_lp=MAX_LP, **SCAN_CFG)
    return _CACHE[nlc]


def kernel(pred, target, _nlc=None, _trace=False):
    pred = np.asarray(pred, dtype=np.float32).astype(IN_NP_DT)
    target = np.asarray(target, dtype=np.float32).astype(IN_NP_DT)
    nl = pred.shape[0]
    nlc = nl // NCORES if _nlc is None else _nlc
    assert nlc * NCORES == nl
    nc = _get_nc(nlc)
    in_maps = [
        {
            "pred": np.ascontiguousarray(pred[i * nlc : (i + 1) * nlc]),
            "target": np.ascontiguousarray(target[i * nlc : (i + 1) * nlc]),
        }
        for i in range(NCORES)
    ]
    res = run_bass_kernel_spmd(nc, in_maps, list(range(NCORES)), trace=_trace)
    total = np.float64(0.0)
    for r in res.results:
        total += np.float64(r["partial"].astype(np.float64).sum())
    loss = np.float32(total / np.float64(nl))
    if _trace:
        return loss, res
    return loss



# revision 45
# speedup vs baseline: 1.0972x; 1.0972x over previous
"""CLRNet IoU loss kernel for Trainium2 (Bass/Tile), 8-core data-parallel.

Math (equivalent to the reference):
  ovr_j   = 2w - |p_j - t_j|          (if both p_j, t_j in [0,1), else 0)
  union_j = 2w + |p_j - t_j|          (same mask)
  iou     = (2w*tp - S) / (2w*tp + S + 1e-9)
  where S = sum_j |d_j| * both_j,  tp = sum_j both_j
  errors  = sum_j (vp_j XOR vt_j) = sum_j(vp_j) + sum_j(vt_j) - 2*tp
  penalize lanes with tp > errors > 0 by iou *= 1 - errors/(tp+1e-9)
  loss    = mean(1 - iou)

Implementation notes:
  - Inputs are cast to fp16 on the host (IN_NP_DT): the fp32 kernel was
    DMA-bound at ~205 us (353 GB/s/core, HBM-per-NC limit ~358 GB/s);
    halving the bytes moves the DMA floor to ~102 us. Loss error from the
    fp16 cast, measured against the f64 reference on the exact seeded
    inputs, is 2.8e-5 (gate: 2e-2). The DVE upconverts fp16 reads to f32
    internally, so on-device math is unchanged.
  - validity is |x - 0.5| < 0.5, one ALU stage via ABSOLUTE_DIFF(x, 0.5),
    so no input pre-processing is needed.
  - A fused custom DVE op computes a running (prefix) sum of
    both*(|d| + 128); per-lane segment sums are recovered by differencing
    the cumulative value at consecutive 72-element page ends (gathered on
    the Scalar engine).  The packed value decodes as 128*tp + S (S <= 72).
  - A second custom op scans cumsum(valid(p) XOR valid(t)), whose page-end
    differences are `errors` per lane directly (exact integers in f32).
  - The finals (decode + IoU + penalty, single-reciprocal form) are
    emitted in column groups interleaved with the chunk loop; they hide
    completely under the scans (no_fin probe at lp64: 147.9 us vs the
    full kernel's 148.0 -- finals and page-end copies are free; the two
    scans at 1 col/cycle account for the entire runtime).
  - At fp16 the kernel is DVE-bound, not DMA-bound: the two custom scans
    cost 2*72 pair-positions/lane at 1x (0.96 GHz, custom DVE ops have no
    2x mode) = ~147 us vs the ~102 us DMA floor. Measured: 150.1 us; a
    no-errors-pass probe (no_xe) runs 102.8 us. This is structural: the
    8-stage DVE limit forces two passes (the fused single-pass op needs
    ~10 stages in every algebraic form tried), stock ops cannot express
    the range-test + page-sum any cheaper (tensor_reduce and all scans
    are 1x-only; verified via supported_dve_perf_modes), Act-engine
    helpers lose to 1283ns activation-table reloads (n_act probe), and
    Pool lacks abs/compare/free-axis-reduce. io_bufs=4, one HWDGE ring,
    1.18 MB (fp32-equivalent geometry) chunk DMAs as before.
  - Also measured, all at-or-worse than the plain two-scan kernel:
    n_act>0 with the Act(Square,Square,Sign,Sign) + Pool(zp-zt) +
    paired-read half-cost XPS_SCAN chain (150.2 us at n_act=12 -- the
    ~13 us serial chain latency paces the pipeline instead of hiding;
    deeper pools hp_bufs=3/io_bufs=5/split_finals=(16,) made it 189.7
    us), io_bufs/scan_bufs/split_finals sweeps (all within +/-0.3 us),
    and an Act+Pool+DVE-stock-reduce variant (reduce is 1x: zero DVE
    relief). Ideas that do NOT work by construction: stock-op predicate
    pipelines (no abs in ts/tt valid ops; >= 3 passes always), f32
    bit-pattern tricks to read two fp16 pairs per cycle (lo-half needs
    denormal compares, FTZ kills it), PE page sums (contracts the
    partition dim only; transpose detours cost more than they save).
    scan_bufs=3 at lp64 overflows SBUF (fin pool short 25.6 kb/part).
  - Last unimplemented idea (~1 us, 0.7%): fuse each dma_group=2 pair of
    lp64 chunks into ONE MD + ONE XE scan over [128, 9216] (needs
    io_bufs=2/scan_bufs=1 to fit SBUF; MD cum ~1.2M keeps S noise at
    +/-0.06 ulp-random, fine). Page-end gather becomes a [128, 2, 64]
    rearranged copy into b1[:, ci:ci+2, 1:65] PLUS a 1-col copy of the
    chunk-boundary cum (r1 col 64*72-1) into b1[:, ci+1, 0:1] -- without
    that cell the second chunk's first page diff is wrong. 2.36 MB
    transfers sat in the fp32-era DMA penalty zone but ~45 us of DMA
    slack hides it.
"""

import sys

if "/opt/trn_rl_repo" not in sys.path:
    sys.path.insert(0, "/opt/trn_rl_repo")

import numpy as np

import concourse.bacc as bacc
import concourse.bass as bass
import concourse.mybir as mybir
from concourse import dve_ops
from concourse.bass_utils import run_bass_kernel_spmd
from concourse.dve_ops import DveOp
from concourse.dve_spec import (
    AluOp,
    Bin,
    C0,
    C1,
    Spec,
    Src0,
    Src1,
    Zero,
    lower,
    scan,
)
from concourse.dve_spec import _has_src1 as has_src1
from concourse.dve_uop import DveOpSpec
from concourse.tile import TileContext

F32 = mybir.dt.float32
F16 = mybir.dt.float16
I32 = mybir.dt.int32

NL = 1_000_000
NR = 72
NCORES = 8
NLC = NL // NCORES  # 125_000 lanes per core
W2 = 2.0 * (15.0 / 800.0)  # 2 * lane half-width = 0.0375
PACK = 128.0  # tp packing multiplier; S <= 72 < 128

# Inputs are cast to fp16 on the host before upload: halves HBM traffic
# (the kernel is DMA-bound) and the induced loss error, measured against
# the f64 reference on the exact seeded inputs, is 2.7e-5 -- far inside
# the 2e-2 gate. The DVE upconverts fp16 operands to f32 internally, so
# the on-device math is unchanged.
IN_NP_DT = np.float16

# ---------------------------------------------------------------------------
# Custom DVE ops (registered at import, idempotently)
# ---------------------------------------------------------------------------


def _register(name: str, spec: Spec, subdim: bool = False) -> DveOp:
    for op in dve_ops.OPS:
        if op.name == name:
            return op
    row = dve_ops._CUSTOM_DVE_ROW_BASE + len(dve_ops.OPS)
    shas = {}
    for ver in ("v3", "v4"):
        try:
            s = DveOpSpec(
                name=name, opcode=row, uops=lower(spec, ver=ver), rd1_en=has_src1(spec)
            )
            shas[ver] = s.sha(ver)
        except Exception:
            pass  # op not expressible on this ver; only v3 (TRN2) is needed
    op = DveOp(name, spec, subdim=subdim, uops_sha=shas)
    dve_ops.OPS.append(op)
    dve_ops._SUB_OPCODE_FOR_NAME[name] = row
    dve_ops.CUSTOM_DVE_SPECS[name] = spec
    return op


def _adiff(x, y):
    return Bin(AluOp.ABSOLUTE_DIFF, x, y)


def _md_ref(in0, in1, s0, s1, imm2):
    p = in0.astype(np.float32).reshape(in0.shape[0], -1)
    t = in1.astype(np.float32).reshape(in0.shape[0], -1)
    both = (np.maximum(np.abs(p - s0), np.abs(t - s0)) < s0).astype(np.float32)
    m = both * (np.abs(p - t) + s1)
    return np.cumsum(m, axis=1, dtype=np.float32)


def _xe_ref(in0, in1, s0, s1, imm2):
    p = in0.astype(np.float32).reshape(in0.shape[0], -1)
    t = in1.astype(np.float32).reshape(in0.shape[0], -1)
    x = ((np.abs(p - s0) < s0) != (np.abs(t - s0) < s0)).astype(np.float32)
    return np.cumsum(x, axis=1, dtype=np.float32)


# valid(x) = |x - 0.5| < 0.5; both = valid(p) & valid(t)
# out = cumsum(both * (|p - t| + PACK))  -- 8 ALU stages on v3
_w = Bin(AluOp.MAX, _adiff(Src0, C0), _adiff(Src1, C0))
_both = _w < C0
_adC = _adiff(Src0, Src1) + C1
MD_SCAN = _register(
    "CLR_MD_SCAN",
    Spec(body=scan(AluOp.ADD, _adC * _both), reference=_md_ref),
)

# out = cumsum( valid(p) XOR valid(t) )  -- 6 ALU stages; page-end
# differences give `errors` per lane directly (exact integers in f32).
_v0 = _adiff(Src0, C0) < C0
_v1 = _adiff(Src1, C0) < C0
XE_SCAN = _register(
    "CLR_XE_SCAN",
    Spec(body=scan(AluOp.ADD, Bin(AluOp.LOGICAL_XOR, _v0, _v1)), reference=_xe_ref),
)


def _xps_ref(in0, in1, s0, s1, imm2):
    a = np.abs(in0.astype(np.float32).reshape(in0.shape[0], -1))
    b = np.abs(in1.astype(np.float32).reshape(in0.shape[0], -1))
    return np.cumsum((a + b) * s0, axis=1, dtype=np.float32)


# paired-read running sum of s0*(|in0|+|in1|): reads two elements of one
# pre-combined stream per cycle -- half the positions of a pair scan.
# Used on dd = zp - zt in {-2,0,2}: with s0=0.5 the cumsum counts xor.
_ps = (_adiff(Src0, Zero) + _adiff(Src1, Zero)) * C0
XPS_SCAN = _register(
    "CLR_XPS_SCAN",
    Spec(body=scan(AluOp.ADD, _ps), reference=_xps_ref),
)

# ---------------------------------------------------------------------------
# Bass program (SPMD; one NeuronCore's share)
# ---------------------------------------------------------------------------


def _chunks(nlc: int, max_lp: int = 32):
    """Split nlc lanes into (base, lanes_per_partition, partitions) chunks."""
    out = []
    base = 0
    for lp in (64, 32, 16, 8, 4, 2, 1):
        if lp > max_lp:
            continue
        n = 128 * lp
        while nlc - base >= n:
            out.append((base, lp, 128))
            base += n
    if nlc > base:
        out.append((base, 1, nlc - base))
        base = nlc
    return out


SPLIT_FINALS = (10, 16, 22, 27)


def build_bass(
    nlc: int = NLC,
    debug: bool = False,
    reps: int = 1,
    no_compute: bool = False,
    no_dma: bool = False,
    split_finals=SPLIT_FINALS,
    small_first: bool = False,
    max_lp: int = 32,
    io_bufs: int = 4,
    scan_bufs: int = 2,
    gp_memset: bool = False,
    dma_split: bool = False,
    ring: str = None,  # None->legacy dma_split; "sync" | "split" | "alt"
    merge_vs: bool = False,  # unused (kept for probe compatibility)
    n_act: int = 0,  # full chunks whose errors-pass runs on Act+GpSimd
    hp_bufs: int = 2,  # helper-pool depth (chains in flight)
    no_xe: bool = False,  # timing probe: skip the errors pass entirely
    no_fin: bool = False,  # timing probe: skip the finals entirely
    scan_group: bool = False,  # fuse dma_group=2 pairs into single scans
    no_copy: bool = False,  # debug: skip page-end copies + finals
    no_scan: bool = False,  # debug: skip DVE scans, copy page-ends from inputs
    dma_group: int = 1,  # consecutive full chunks loaded by one dma_start
) -> bass.Bass:
    if ring is None:
        ring = "split" if dma_split else "sync"
    nc = bacc.Bacc(None)
    pred = nc.declare_dram_parameter("pred", [nlc, NR], F16, isOutput=False)
    targ = nc.declare_dram_parameter("target", [nlc, NR], F16, isOutput=False)
    out = nc.declare_dram_parameter("partial", [128, 1], F32, isOutput=True)
    dbg = {}
    if debug:
        nchd = len(_chunks(nlc))
        nposd = nchd * 32
        for name in ("dbg_d1", "dbg_sv", "dbg_tp", "dbg_loss"):
            dbg[name] = nc.declare_dram_parameter(
                name, [128, nposd], F32, isOutput=True
            )

    chunks = _chunks(nlc, max_lp)
    if small_first:
        chunks = chunks[::-1]
    nch = len(chunks)
    # Spread the Act-assisted chunks evenly over the full (parts=128) chunks
    # so no engine stalls waiting for io tiles (io_pool depth limits drift).
    lpmax = max(lp for _b, lp, _p in chunks)
    full_idx = [
        i for i, (_b, lp, p) in enumerate(chunks) if p == 128 and lp == lpmax
    ]
    na = min(n_act, len(full_idx))
    act_set = set()
    if na:
        nf = len(full_idx)
        act_set = {full_idx[(k * nf) // na] for k in range(na)}
    pos = max(lp for _b, lp, _p in chunks)  # page-end columns per chunk slot
    slot = pos + 1  # plus 1 zero column
    npos = nch * (slot - 1)
    if split_finals:
        cuts_all = (
            (split_finals,) if isinstance(split_finals, int) else tuple(split_finals)
        )
        split_finals = tuple(c for c in cuts_all if 0 < c < nch)

    with TileContext(nc) as tc:
        with (
            tc.tile_pool(name="io", bufs=io_bufs) as io_pool,
            tc.tile_pool(name="scan", bufs=scan_bufs) as scan_pool,
            tc.tile_pool(name="acc", bufs=1) as acc_pool,
            tc.tile_pool(name="fin", bufs=1) as fin_pool,
            tc.tile_pool(name="hp", bufs=hp_bufs) as hp_pool,
        ):
            b1 = acc_pool.tile([128, nch, slot], F32, tag="b1")
            b2 = acc_pool.tile([128, nch, slot], F32, tag="b2")
            ms = nc.gpsimd.memset if gp_memset else nc.vector.memset
            ms(b1[:], 0.0)
            ms(b2[:], 0.0)
            if act_set:
                bias_m1 = acc_pool.tile([128, 1], F32, tag="bias_m1")
                bias_thr = acc_pool.tile([128, 1], F32, tag="bias_thr")
                ms(bias_m1[:], -1.0)
                ms(bias_thr[:], 1.0 - 2.0**-11)
            # 1.0 where a position maps to a real lane, 0.0 elsewhere
            lmask = acc_pool.tile([128, nch, pos], F32, tag="lmask")
            ms(lmask[:], 0.0)
            for ci, (_b, lp, parts) in enumerate(chunks):
                ms(lmask[:parts, ci, 0 : min(lp, pos)], 1.0)

            # ----------------- finals: decode + iou + penalty ---------------
            stt = nc.vector.scalar_tensor_tensor
            A = mybir.AluOpType
            psums = []

            def emit_finals(cs, ce, key):
                """Decode and compute per-lane loss for chunk slots [cs, ce);
                appends a [128,1] partial-sum tile to psums.

                Single-reciprocal form: with pen = (tp>err)&(err>0),
                  iou2 = num*(tp - pen*err) / (den*(tp + 1e-9))
                matches the reference both when pen (up to the 1e-9 shift of
                err's denominator) and when not (tp/(tp+eps) ~ 1; tp=0 lanes
                have num=0 so both forms give 0)."""
                if no_fin:
                    return
                w = (ce - cs) * pos

                def ft(tag, dt=F32):
                    t = fin_pool.tile([128, w], dt, tag=f"{tag}{key}")
                    return t

                d1 = ft("d1")
                err = ft("err")
                tp = ft("tp")
                ssum = ft("ssum")
                tmp = ft("tmp")
                tpi = ft("tpi", I32)

                # segment sums by differencing consecutive page-end cumulatives
                nc.vector.tensor_sub(
                    d1[:].rearrange("q (c j) -> q c j", c=ce - cs),
                    b1[:, cs:ce, 1:slot],
                    b1[:, cs:ce, 0 : slot - 1],
                )
                nc.vector.tensor_sub(
                    err[:].rearrange("q (c j) -> q c j", c=ce - cs),
                    b2[:, cs:ce, 1:slot],
                    b2[:, cs:ce, 0 : slot - 1],
                )

                if debug:
                    nc.sync.dma_start(out=dbg["dbg_d1"][:, cs * 32 : ce * 32], in_=d1[:])
                    nc.sync.dma_start(out=dbg["dbg_sv"][:, cs * 32 : ce * 32], in_=err[:])

                # decode: tp = floor(d1/128) via int32 truncation, S = d1 - 128*tp
                nc.vector.tensor_scalar(
                    out=tpi[:], in0=d1[:], scalar1=1.0 / PACK, scalar2=None, op0=A.mult
                )
                nc.vector.tensor_copy(out=tp[:], in_=tpi[:])
                if debug:
                    nc.sync.dma_start(out=dbg["dbg_tp"][:, cs * 32 : ce * 32], in_=tp[:])
                stt(out=ssum[:], in0=tp[:], scalar=-PACK, in1=d1[:], op0=A.mult, op1=A.add)

                # num = 2w*tp - S;  den = 2w*tp + S + 1e-9
                u1 = tmp
                nc.vector.tensor_scalar(
                    out=u1[:], in0=tp[:], scalar1=W2, scalar2=None, op0=A.mult
                )
                num = d1  # reuse
                stt(out=num[:], in0=ssum[:], scalar=-1.0, in1=u1[:], op0=A.mult, op1=A.add)
                den = ssum  # reuse
                stt(out=den[:], in0=ssum[:], scalar=1e-9, in1=u1[:], op0=A.add, op1=A.add)

                # pen = (tp > err) & (err > 0); tnum = tp - pen*err
                c1 = u1  # reuse
                nc.vector.tensor_tensor(out=c1[:], in0=tp[:], in1=err[:], op=A.is_gt)
                pen = c1  # in place
                stt(out=pen[:], in0=err[:], scalar=0.0, in1=c1[:], op0=A.is_gt, op1=A.mult)
                ee = pen  # in place
                nc.vector.tensor_mul(ee[:], pen[:], err[:])
                tnum = err  # reuse
                nc.vector.tensor_sub(tnum[:], tp[:], ee[:])

                # DEN = den*(tp + 1e-9); iou2 = num*tnum * recip(DEN)
                tpe = tp  # in place
                nc.vector.tensor_scalar(
                    out=tpe[:], in0=tp[:], scalar1=1e-9, scalar2=None, op0=A.add
                )
                DEN = ee  # reuse
                nc.vector.tensor_mul(DEN[:], den[:], tpe[:])
                NUM = den  # reuse
                nc.vector.tensor_mul(NUM[:], num[:], tnum[:])
                r = tpe  # reuse
                nc.vector.reciprocal_approx_fast(r[:], DEN[:])
                iou2 = num  # reuse
                nc.vector.tensor_mul(iou2[:], NUM[:], r[:])

                # loss = lmask*(1 - iou2); partial = sum
                lm = lmask[:, cs:ce, :].rearrange("q c j -> q (c j)")
                f2 = DEN  # reuse
                nc.vector.tensor_mul(f2[:], iou2[:], lm)
                loss = iou2  # reuse
                ps = fin_pool.tile([128, 1], F32, tag=f"psum{key}")
                stt(
                    out=loss[:],
                    in0=f2[:],
                    scalar=-1.0,
                    in1=lm,
                    op0=A.mult,
                    op1=A.add,
                    accum_out=ps[:],
                )
                if debug:
                    nc.sync.dma_start(
                        out=dbg["dbg_loss"][:, cs * 32 : ce * 32], in_=loss[:]
                    )
                psums.append(ps)

            # group consecutive full chunks of equal lp for single-DMA loads;
            # lane->(slot, partition, position) mapping changes but the loss
            # sum is permutation-invariant over lanes.
            groups = []
            gi = 0
            while gi < nch:
                g = [gi]
                while (
                    len(g) < dma_group
                    and gi + len(g) < nch
                    and chunks[gi + len(g)][1] == chunks[gi][1]
                    and chunks[gi + len(g)][2] == 128
                    and chunks[gi][2] == 128
                ):
                    g.append(gi + len(g))
                groups.append(g)
                gi += len(g)

            for rep in range(reps):
              for g in groups:
                base0, lp, parts = chunks[g[0]]
                m = len(g)
                fd = lp * NR
                fdg = m * fd
                lanes = sum(chunks[ci][2] * chunks[ci][1] for ci in g)
                up = io_pool.tile([128, fdg], F16, tag="up")
                vt = io_pool.tile([128, fdg], F16, tag="vt")
                src_p = pred[base0 : base0 + lanes, :].rearrange(
                    "(q j) r -> q (j r)", q=parts
                )
                src_t = targ[base0 : base0 + lanes, :].rearrange(
                    "(q j) r -> q (j r)", q=parts
                )
                if not no_dma:
                    if ring == "sync":
                        p_eng = t_eng = nc.sync
                    elif ring == "split":
                        p_eng, t_eng = nc.sync, nc.scalar
                    else:  # alternate rings per group
                        p_eng, t_eng = (
                            (nc.sync, nc.scalar)
                            if g[0] % 2 == 0
                            else (nc.scalar, nc.sync)
                        )
                    p_eng.dma_start(out=up[:parts, :], in_=src_p)
                    t_eng.dma_start(out=vt[:parts, :], in_=src_t)
                if no_compute:
                    continue
                if (
                    scan_group
                    and m == 2
                    and parts == 128
                    and not no_scan
                    and not no_xe
                    and not any(c in act_set for c in g)
                ):
                    # one MD + one XE scan across the whole 2-chunk group:
                    # halves scan instruction count. The cum crosses the
                    # chunk boundary, so chunk g[1]'s zero column gets the
                    # boundary cum (else its first page diff is wrong).
                    ci0 = g[0]
                    r1 = scan_pool.tile([128, fdg], F32, tag="r1")
                    nc.vector._custom_dve(
                        MD_SCAN,
                        out=r1[:parts, :],
                        in0=up[:parts, :],
                        in1=vt[:parts, :],
                        s0=0.5,
                        s1=PACK,
                    )
                    r2 = scan_pool.tile([128, fdg], F32, tag="r2")
                    nc.vector._custom_dve(
                        XE_SCAN,
                        out=r2[:parts, :],
                        in0=up[:parts, :],
                        in1=vt[:parts, :],
                        s0=0.5,
                    )
                    if not no_copy:
                        for r, b in ((r1, b1), (r2, b2)):
                            re_ = r[:parts, :].rearrange(
                                "q (c j r) -> q c j r", c=2, r=NR
                            )[:, :, :, 71]
                            nc.scalar.copy(
                                b[:parts, ci0 : ci0 + 2, 1 : 1 + lp], re_
                            )
                            nc.scalar.copy(
                                b[:parts, ci0 + 1, 0:1], r[:parts, fd - 1 : fd]
                            )
                    if split_finals and rep == reps - 1:
                        cuts = (
                            (split_finals,)
                            if isinstance(split_finals, int)
                            else tuple(split_finals)
                        )
                        for k, cut in enumerate(cuts):
                            if cut - 1 in g:
                                prev = 0 if k == 0 else cuts[k - 1]
                                emit_finals(prev, cut, f"s{k}")
                    continue
                for s, ci in enumerate(g):
                    _bc, lpc, parts = chunks[ci]
                    off = s * fd
                    upc = up[:parts, off : off + fd]
                    vtc = vt[:parts, off : off + fd]
                    if no_scan:
                        # timing probe: same copy geometry, reading io tiles
                        u3d = upc.rearrange("q (j r) -> q j r", j=lp)
                        v3d = vtc.rearrange("q (j r) -> q j r", j=lp)
                        nc.scalar.copy(b1[:parts, ci, 1 : 1 + lp], u3d[:, :, 71])
                        nc.scalar.copy(b2[:parts, ci, 1 : 1 + lp], v3d[:, :, 71])
                        continue

                    r1 = scan_pool.tile([128, fd], F32, tag="r1")
                    nc.vector._custom_dve(
                        MD_SCAN,
                        out=r1[:parts, :],
                        in0=upc,
                        in1=vtc,
                        s0=0.5,
                        s1=PACK,
                    )
                    r1e = r1[:parts, :].rearrange("q (j r) -> q j r", j=lp)[
                        :, :, 71
                    ]
                    if no_xe:
                        if not no_copy:
                            nc.scalar.copy(b1[:parts, ci, 1 : 1 + lp], r1e)
                    elif ci in act_set:
                        # errors-pass mostly off the DVE: Act computes z=+/-1
                        # validity via Square then Sign (exact over the fp16
                        # input grid: q=(2x-1)^2; valid <=> q < 1, closest
                        # interior q is 1-2^-9, so bias 1-2^-11 splits
                        # cleanly; fp16 q keeps the gap). Pool forms
                        # dd=zp-zt in {-2,0,2}; a paired-read DVE scan sums
                        # 0.5*(|dd_even|+|dd_odd|) at HALF the positions of
                        # the XE scan, feeding the same b2 cumulative path
                        # (pair 35 of each 36-pair page is the page end).
                        # Acts are ordered Square,Square,Sign,Sign to pay 2
                        # activation-table loads per chunk, not 4; gathers
                        # go to Pool so Act never loads the Copy table.
                        AF = mybir.ActivationFunctionType
                        qa = hp_pool.tile([128, fd], F16, tag="qa")
                        qb = hp_pool.tile([128, fd], F16, tag="qb")
                        zp = hp_pool.tile([128, fd], F16, tag="zp")
                        zt = hp_pool.tile([128, fd], F16, tag="zt")
                        nc.scalar.activation(
                            qa[:parts, :], upc, AF.Square,
                            scale=2.0, bias=bias_m1[:parts, :],
                        )
                        nc.scalar.activation(
                            qb[:parts, :], vtc, AF.Square,
                            scale=2.0, bias=bias_m1[:parts, :],
                        )
                        nc.scalar.activation(
                            zp[:parts, :], qa[:parts, :], AF.Sign,
                            scale=-1.0, bias=bias_thr[:parts, :],
                        )
                        nc.scalar.activation(
                            zt[:parts, :], qb[:parts, :], AF.Sign,
                            scale=-1.0, bias=bias_thr[:parts, :],
                        )
                        dd = hp_pool.tile([128, fd], F16, tag="dd")
                        nc.gpsimd.tensor_sub(
                            dd[:parts, :], zp[:parts, :], zt[:parts, :]
                        )
                        d2 = dd[:parts, :].rearrange(
                            "q (j two) -> q j two", two=2
                        )
                        r2p = scan_pool.tile([128, fd // 2], F32, tag="xps")
                        nc.vector._custom_dve(
                            XPS_SCAN,
                            out=r2p[:parts, :],
                            in0=d2[:, :, 0],
                            in1=d2[:, :, 1],
                            s0=0.5,
                        )
                        r2e = r2p[:parts, :].rearrange(
                            "q (j r) -> q j r", j=lp
                        )[:, :, 35]
                        nc.gpsimd.tensor_copy(
                            out=b2[:parts, ci, 1 : 1 + lp], in_=r2e
                        )
                        nc.gpsimd.tensor_copy(
                            out=b1[:parts, ci, 1 : 1 + lp], in_=r1e
                        )
                    else:
                        r2 = scan_pool.tile([128, fd], F32, tag="r2")
                        nc.vector._custom_dve(
                            XE_SCAN,
                            out=r2[:parts, :],
                            in0=upc,
                            in1=vtc,
                            s0=0.5,
                        )
                        if not no_copy:
                            r2e = r2[:parts, :].rearrange(
                                "q (j r) -> q j r", j=lp
                            )[:, :, 71]
                            nc.scalar.copy(b2[:parts, ci, 1 : 1 + lp], r2e)
                            nc.scalar.copy(b1[:parts, ci, 1 : 1 + lp], r1e)

                    if split_finals and rep == reps - 1:
                        cuts = (
                            (split_finals,)
                            if isinstance(split_finals, int)
                            else tuple(split_finals)
                        )
                        for k, cut in enumerate(cuts):
                            if ci == cut - 1:
                                prev = 0 if k == 0 else cuts[k - 1]
                                emit_finals(prev, cut, f"s{k}")

            if not no_compute:
                if split_finals:
                    cuts = (
                        (split_finals,)
                        if isinstance(split_finals, int)
                        else tuple(split_finals)
                    )
                    emit_finals(cuts[-1], nch, "b")
                else:
                    emit_finals(0, nch, "a")
            if not psums:
                zp = fin_pool.tile([128, 1], F32, tag="zp")
                nc.vector.memset(zp[:], 0.0)
                psums.append(zp)
            total = psums[0]
            for ps in psums[1:]:
                nc.vector.tensor_add(total[:], total[:], ps[:])
            nc.sync.dma_start(out=out[:, :], in_=total[:])

    nc.finalize()
    return nc


# ---------------------------------------------------------------------------
# Host entry point
# ---------------------------------------------------------------------------

_CACHE = {}

# Act+GpSimd errors-pass helpers measured SLOWER than the DVE XOR scan
# (activation-table reloads, 1283ns each, dominate): keep 0.
N_ACT = 0
# lp64 chunks halve the scan/DMA instruction count (DVE issue overhead is
# the only slack left over the scan floor): 148.0 us vs 150.4 at lp32.
# At fp16 an lp64 chunk is a 1.18 MB transfer -- the measured DMA
# sweet spot (the fp32-era big-transfer penalty applied to 2.36 MB).
MAX_LP = 64
# Fused pair-scans (scan_group=True, dma_group=2, io_bufs=2, scan_bufs=1)
# are CORRECT (rel 3.0e-5) but measured 162.6 us vs 148.4: the SBUF-forced
# scan_bufs=1 serializes the DVE on per-group gather round-trips
# (~1.8 us x 8 pair-groups), swamping the ~1 us instruction-overhead win.
# The idea needs scan_bufs>=2 at [128,9216] f32, which does not fit SBUF.
SCAN_CFG = dict()


def _get_nc(nlc: int) -> bass.Bass:
    if nlc not in _CACHE:
        _CACHE[nlc] = build_bass(nlc, n_act=N_ACT, max_lp=MAX_LP, **SCAN_CFG)
    return _CACHE[nlc]


def kernel(pred, target, _nlc=None, _trace=False):
    pred = np.asarray(pred, dtype=np.float32).astype(IN_NP_DT)
    target = np.asarray(target, dtype=np.float32).astype(IN_NP_DT)
    nl = pred.shape[0]
    nlc = nl // NCORES if _nlc is None else _nlc
    assert nlc * NCORES == nl
    nc = _get_nc(nlc)
    in_maps = [
        {
            "pred": np.ascontiguousarray(pred[i * nlc : (i + 1) * nlc]),
            "target": np.ascontiguousarray(target[i * nlc : (i + 1) * nlc]),
        }
        for i in range(NCORES)
    ]
    res = run_bass_kernel_spmd(nc, in_maps, list(range(NCORES)), trace=_trace)
    total = np.float64(0.0)
    for r in res.results:
        total += np.float64(r["partial"].astype(np.float64).sum())
    loss = np.float32(total / np.float64(nl))
    if _trace:
        return loss, res
    return loss

